# revision 1
# baseline (speedup 1.0000x reference)
"""Fused attention-block kernel for Trainium2, 8-core data-parallel over batch.

Computation (see harness reference): three BN+ReLU linear branches from the
same input, attention (QK^T/16 -> softmax -> AV), then a fourth BN+ReLU
linear.  BatchNorm1d is training-mode per-channel over (batch, feature) with
channel = sequence position, so batch-sharding needs a cross-core stats
all-reduce (sync-BN); weights are replicated.

Hardcoded: B=256, N=256, D=256, 8 cores -> 32 batches (8192 tokens) per core.
"""
import sys
import types

sys.path.insert(0, "/opt/trn_rl_repo")

import numpy as np
import ml_dtypes
from contextlib import ExitStack

import concourse.bass as bass
import concourse.mybir as mybir
import concourse.tile as tile
from concourse.masks import make_identity

BF16 = mybir.dt.bfloat16
F32 = mybir.dt.float32
NCORES = 8
B_LOC = 32          # batches per core
T = B_LOC * 256     # tokens per core
EPS = 1e-5


def _install_profile_shim():
    """run_bass_kernel_spmd(trace=True) under axon needs antenv.axon_hooks,
    which this image lacks; synthesize it (harmless if tracing unused)."""
    if "antenv.axon_hooks" in sys.modules:
        return
    try:
        import antenv
        mod = types.ModuleType("antenv.axon_hooks")
        mod._hook = None
        mod.set_axon_ntff_profile_hook = lambda h: setattr(mod, "_hook", h)
        mod.get_axon_ntff_profile_hook = lambda: mod._hook
        sys.modules["antenv.axon_hooks"] = mod
        antenv.axon_hooks = mod
        from trn_agent_boot.trn_boot import _ntff_profile_via_ctypes
        hook = _ntff_profile_via_ctypes("/opt/axon/libaxon_pjrt.so")
        if hook is not None:
            mod.set_axon_ntff_profile_hook(hook)
    except Exception:
        pass


def _legalize_waits(nc, max_waits=1):
    """HW instructions carry one sync-wait slot; walrus rejects instructions
    with too many waits.  Hoist extras onto engine-matched NoOps."""
    for f in nc.m.functions:
        for bb in f.blocks:
            insts = bb.instructions
            new_list = []
            for inst in insts:
                si = inst.sync_info
                if si is not None and len(si.on_wait) > max_waits:
                    waits = list(si.on_wait)
                    extra, keep = waits[:-max_waits], waits[-max_waits:]
                    for j, w in enumerate(extra):
                        nop = mybir.InstNoOp(
                            name=f"{inst.name}-waitnop{j}",
                            engine=inst.engine,
                            ins=[], outs=[],
                            sync_info=mybir.SyncInfo(on_wait=[w], on_update=[]),
                        )
                        nc.register_instruction(nop, overwrite=True)
                        new_list.append(nop)
                    inst.sync_info = mybir.SyncInfo(
                        on_wait=keep, on_update=list(si.on_update))
                new_list.append(inst)
            del insts[:]
            for x in new_list:
                insts.append(x)


def build_program(use_collectives=True, pool_compute=False):
    """pool_compute: run casts/relus that fall in collective-free windows on
    the GpSimd (Pool) engine.  Pool compute CONCURRENT with a collective
    hangs the device, so only ops strictly before the first or between/after
    collectives may go there."""
    nc = bass.Bass("TRN2", target_bir_lowering=False, debug=False,
                   num_devices=NCORES)
    GPC = nc.gpsimd if pool_compute else nc.vector

    x_d = nc.dram_tensor("x", [T, 256], F32, kind="ExternalInput")
    w123_d = nc.dram_tensor("w123", [128, 2, 768], BF16, kind="ExternalInput")
    w4_d = nc.dram_tensor("w4", [128, 2, 260], BF16, kind="ExternalInput")
    bb_d = nc.dram_tensor("bb", [128, 4, 256], BF16, kind="ExternalInput")
    gb_d = nc.dram_tensor("gb", [128, 2, 2], F32, kind="ExternalInput")
    hc_d = nc.dram_tensor("hc", [128, 8], F32, kind="ExternalInput")
    out_d = nc.dram_tensor("out", [T, 256], F32, kind="ExternalOutput")

    groups = [list(range(NCORES))]
    x_r = x_d.ap().rearrange("(b h p) e -> p b h e", b=B_LOC, h=2, p=128)
    out_r = out_d.ap().rearrange("(b h p) e -> p b h e", b=B_LOC, h=2, p=128)

    with ExitStack() as ctx:
        tc = ctx.enter_context(tile.TileContext(nc))
        big = ctx.enter_context(tc.tile_pool(name="big", bufs=1))
        small = ctx.enter_context(tc.tile_pool(name="small", bufs=1))
        stage = ctx.enter_context(tc.tile_pool(name="stage", bufs=3))
        att = ctx.enter_context(tc.tile_pool(name="att", bufs=4))
        ps = ctx.enter_context(tc.tile_pool(name="ps", bufs=2, space="PSUM"))
        dram = ctx.enter_context(tc.tile_pool(name="dram", bufs=1, space="DRAM"))

        # ---- constant loads -------------------------------------------------
        w123 = small.tile([128, 2, 768], BF16, tag="w123")
        w4 = small.tile([128, 2, 260], BF16, tag="w4")
        bbt = small.tile([128, 4, 256], BF16, tag="bbt")
        gbt = small.tile([128, 2, 2], F32, tag="gbt")
        hct = small.tile([128, 8], F32, tag="hct")
        idn = small.tile([128, 128], BF16, tag="idn")
        nc.sync.dma_start(out=w123[:], in_=w123_d.ap())
        nc.sync.dma_start(out=w4[:], in_=w4_d.ap())
        nc.sync.dma_start(out=bbt[:], in_=bb_d.ap())
        nc.sync.dma_start(out=gbt[:], in_=gb_d.ap())
        nc.sync.dma_start(out=hct[:], in_=hc_d.ap())
        make_identity(nc, idn[:])

        # ---- x -> bf16 in DRAM (SWDGE cast), then xbar-transposed loads ----
        xT = big.tile([128, 2, T], BF16, tag="tp1")          # (dchunk, token)
        xbf_d = dram.tile([T, 256], BF16, tag="xbf_d")
        NXC = 8   # cast chunks
        for c in range(NXC):
            nc.gpsimd.dma_start(out=xbf_d[c * (T // NXC):(c + 1) * (T // NXC), :],
                                in_=x_d.ap()[c * (T // NXC):(c + 1) * (T // NXC), :])
        NTC = 8   # transpose chunks per d-half
        for dc in range(2):
            for c in range(NTC):
                t0, t1 = c * (T // NTC), (c + 1) * (T // NTC)
                nc.sync.dma_start_transpose(
                    out=xT[:, dc, t0:t1],
                    in_=xbf_d[t0:t1, dc * 128:(dc + 1) * 128])

        # ---- helper: per-layer BN scale/shift from all-reduced stats --------
        def bn_finalize(lidx, artot, wterm=None):
            """artot [128,4] per half h: cols 2h = sum-of-core-means,
            2h+1 = sum-of-core-E[y^2].  Returns (s fp32, bst bf16) tiles."""
            meany = small.tile([128, 2], F32, tag=f"meany{lidx}", name=f"my{lidx}")
            ey2 = small.tile([128, 2], F32, tag=f"ey2{lidx}", name=f"ey{lidx}")
            nc.vector.tensor_scalar_mul(meany[:], artot[:, 0:4:2], 1.0 / NCORES)
            nc.vector.tensor_scalar_mul(ey2[:], artot[:, 1:4:2], 1.0 / NCORES)
            meanz = small.tile([128, 2], F32, tag=f"meanz{lidx}", name=f"mz{lidx}")
            nc.vector.tensor_scalar_add(meanz[:], meany[:], hct[:, lidx:lidx + 1])
            varz = small.tile([128, 2], F32, tag=f"varz{lidx}", name=f"vz{lidx}")
            m2 = small.tile([128, 2], F32, tag=f"m2_{lidx}", name=f"m2{lidx}")
            if wterm is not None:
                # exact: E[z^2] = E[y^2] + 2 E[y b] + mean(b^2)
                eyb = small.tile([128, 2], F32, tag=f"eyb{lidx}", name=f"eb{lidx}")
                nc.vector.tensor_scalar_mul(eyb[:], wterm[:], 2.0 / 65536.0)
                nc.vector.tensor_tensor(out=ey2[:], in0=ey2[:], in1=eyb[:],
                                        op=mybir.AluOpType.add)
                nc.vector.tensor_scalar_add(ey2[:], ey2[:], hct[:, 7:8])
                nc.vector.tensor_tensor(out=m2[:], in0=meanz[:], in1=meanz[:],
                                        op=mybir.AluOpType.mult)
                nc.vector.tensor_tensor(out=varz[:], in0=ey2[:], in1=m2[:],
                                        op=mybir.AluOpType.subtract)
                nc.vector.tensor_scalar_add(varz[:], varz[:], EPS)
            else:
                # var_z ~= var_y + var(b) (bias covariance negligible here)
                nc.vector.tensor_tensor(out=m2[:], in0=meany[:], in1=meany[:],
                                        op=mybir.AluOpType.mult)
                nc.vector.tensor_tensor(out=varz[:], in0=ey2[:], in1=m2[:],
                                        op=mybir.AluOpType.subtract)
                nc.vector.tensor_scalar(varz[:], varz[:],
                                        hct[:, 4 + lidx:5 + lidx], EPS,
                                        mybir.AluOpType.add,
                                        mybir.AluOpType.add)
            sd = small.tile([128, 2], F32, tag=f"sd{lidx}", name=f"sd{lidx}")
            nc.scalar.sqrt(out=sd[:], in_=varz[:])
            rstd = small.tile([128, 2], F32, tag=f"rstd{lidx}", name=f"rs{lidx}")
            nc.vector.reciprocal(out=rstd[:], in_=sd[:])
            s = small.tile([128, 2], F32, tag=f"s{lidx}", name=f"s{lidx}")
            nc.vector.tensor_tensor(out=s[:], in0=rstd[:], in1=gbt[:, :, 0],
                                    op=mybir.AluOpType.mult)
            tsh = small.tile([128, 2], F32, tag=f"tsh{lidx}", name=f"t{lidx}")
            nc.vector.tensor_tensor(out=tsh[:], in0=meanz[:], in1=s[:],
                                    op=mybir.AluOpType.mult)
            nc.vector.tensor_tensor(out=tsh[:], in0=gbt[:, :, 1], in1=tsh[:],
                                    op=mybir.AluOpType.subtract)
            bst = small.tile([128, 2, 256], BF16, tag=f"bst{lidx}", name=f"b{lidx}")
            for h in range(2):
                nc.vector.tensor_scalar(bst[:, h, :], bbt[:, lidx, :],
                                        s[:, h:h + 1], tsh[:, h:h + 1],
                                        mybir.AluOpType.mult,
                                        mybir.AluOpType.add)
            return s, bst

        def emit_allreduce(lidx, arin, width):
            ar_i = dram.tile([128, width], F32, tag=f"ari{lidx}", name=f"ai{lidx}")
            ar_o = dram.tile([128, width], F32, tag=f"aro{lidx}", name=f"ao{lidx}")
            nc.sync.dma_start(out=ar_i[:], in_=arin[:])
            if use_collectives:
                nc.gpsimd.collective_compute(
                    "AllReduce", mybir.AluOpType.add, replica_groups=groups,
                    ins=[ar_i[:].opt()], outs=[ar_o[:].opt()])
            else:
                nc.gpsimd.dma_start(out=ar_o[:], in_=ar_i[:])
            artot = small.tile([128, width], F32, tag=f"artot{lidx}",
                               name=f"at{lidx}")
            nc.sync.dma_start(out=artot[:], in_=ar_o[:])
            return artot

        def stats_cols(lidx, stats, arin, col0):
            """stats: [128, B_LOC, 2, 6] per-batch bn_stats rows (each row =
            even/odd 3-tuples (count=128, mean, count*var) x2).  Write per
            half h: arin[:, col0+2h] = core mean, col0+2h+1 = core E[y^2].
            Manual aggregation (equal counts): mean = sum(means)/64,
            E[y^2] = sum(cv)/(128*64) + sum(mean^2)/64."""
            for h in range(2):
                msum = small.tile([128, 1], F32, tag=f"ms{lidx}_{h}",
                                  name=f"ms{lidx}{h}")
                nc.vector.tensor_reduce(out=msum[:], in_=stats[:, :, h, 1:5:3],
                                        axis=mybir.AxisListType.XY,
                                        op=mybir.AluOpType.add)
                sq = small.tile([128, B_LOC, 2], F32, tag=f"sq{lidx}_{h}",
                                name=f"sq{lidx}{h}")
                nc.vector.tensor_tensor(out=sq[:], in0=stats[:, :, h, 1:5:3],
                                        in1=stats[:, :, h, 1:5:3],
                                        op=mybir.AluOpType.mult)
                sqsum = small.tile([128, 1], F32, tag=f"qs{lidx}_{h}",
                                   name=f"qs{lidx}{h}")
                nc.vector.tensor_reduce(out=sqsum[:], in_=sq[:],
                                        axis=mybir.AxisListType.XY,
                                        op=mybir.AluOpType.add)
                cvsum = small.tile([128, 1], F32, tag=f"cv{lidx}_{h}",
                                   name=f"cv{lidx}{h}")
                nc.vector.tensor_reduce(out=cvsum[:], in_=stats[:, :, h, 2:6:3],
                                        axis=mybir.AxisListType.XY,
                                        op=mybir.AluOpType.add)
                nc.vector.tensor_scalar_mul(
                    arin[:, col0 + 2 * h:col0 + 2 * h + 1], msum[:],
                    1.0 / (2 * B_LOC))
                nc.vector.tensor_scalar_mul(cvsum[:], cvsum[:],
                                            1.0 / (128 * 2 * B_LOC))
                nc.vector.tensor_scalar_mul(
                    arin[:, col0 + 2 * h + 1:col0 + 2 * h + 2], sqsum[:],
                    1.0 / (2 * B_LOC))
                nc.vector.tensor_tensor(
                    out=arin[:, col0 + 2 * h + 1:col0 + 2 * h + 2],
                    in0=arin[:, col0 + 2 * h + 1:col0 + 2 * h + 2],
                    in1=cvsum[:], op=mybir.AluOpType.add)

        def stats_cols_il(lidx, st, arin, col0):
            """st: [128, B_LOC, 2, 6]; z1 occupies even elements (cols 0:3),
            z2 odd (cols 3:6) of each interleaved 512-row.  Per half h:
            arin[:, col0+2h] = core mean, +1 = core E[y^2]."""
            for h in range(2):
                msum = small.tile([128, 1], F32, tag=f"ms{lidx}_{h}",
                                  name=f"ms{lidx}{h}")
                nc.vector.tensor_reduce(
                    out=msum[:], in_=st[:, :, h, 3 * lidx + 1:3 * lidx + 2],
                    axis=mybir.AxisListType.XY, op=mybir.AluOpType.add)
                sq = small.tile([128, B_LOC, 1], F32, tag=f"sq{lidx}_{h}",
                                name=f"sq{lidx}{h}")
                nc.vector.tensor_tensor(
                    out=sq[:], in0=st[:, :, h, 3 * lidx + 1:3 * lidx + 2],
                    in1=st[:, :, h, 3 * lidx + 1:3 * lidx + 2],
                    op=mybir.AluOpType.mult)
                sqsum = small.tile([128, 1], F32, tag=f"qs{lidx}_{h}",
                                   name=f"qs{lidx}{h}")
                nc.vector.tensor_reduce(out=sqsum[:], in_=sq[:],
                                        axis=mybir.AxisListType.XY,
                                        op=mybir.AluOpType.add)
                cvsum = small.tile([128, 1], F32, tag=f"cv{lidx}_{h}",
                                   name=f"cv{lidx}{h}")
                nc.vector.tensor_reduce(
                    out=cvsum[:], in_=st[:, :, h, 3 * lidx + 2:3 * lidx + 3],
                    axis=mybir.AxisListType.XY, op=mybir.AluOpType.add)
                nc.vector.tensor_scalar_mul(
                    arin[:, col0 + 2 * h:col0 + 2 * h + 1], msum[:], 1.0 / B_LOC)
                nc.vector.tensor_scalar_mul(cvsum[:], cvsum[:],
                                            1.0 / (256 * B_LOC))
                nc.vector.tensor_scalar_mul(
                    arin[:, col0 + 2 * h + 1:col0 + 2 * h + 2], sqsum[:],
                    1.0 / B_LOC)
                nc.vector.tensor_tensor(
                    out=arin[:, col0 + 2 * h + 1:col0 + 2 * h + 2],
                    in0=arin[:, col0 + 2 * h + 1:col0 + 2 * h + 2],
                    in1=cvsum[:], op=mybir.AluOpType.add)

        # ---- layers 1+2 fused (one 512-wide matmul), then layer 3 -----------
        z12 = big.tile([128, B_LOC, 2, 512], BF16, tag="tpA")   # (b, h, z1|z2)
        st12 = small.tile([128, B_LOC, 2, 6], F32, tag="st12")
        ps2_cm = tc.tile_pool(name="ps2", bufs=3, space="PSUM")
        ps2 = ps2_cm.__enter__()
        for b in range(B_LOC):
            psz = ps2.tile([128, 2, 512], F32, tag="ps2", name=f"pz{b}")
            for h in range(2):
                for dc in range(2):
                    nc.tensor.matmul(
                        out=psz[:, h, :],
                        lhsT=xT[:, dc, b * 256 + h * 128: b * 256 + (h + 1) * 128],
                        rhs=w123[:, dc, 0:512],
                        start=(dc == 0), stop=(dc == 1))
            nc.any.tensor_copy(out=z12[:, b, :, :], in_=psz[:])
            for h in range(2):
                nc.vector.bn_stats(out=st12[:, b, h, :], in_=z12[:, b, h, :])
        ps2_cm.__exit__(None, None, None)
        arin12 = small.tile([128, 8], F32, tag="arin12")
        # st12 rows are (h, l): l-major cols in arin: L1 -> 0..3, L2 -> 4..7
        for l in range(2):
            stats_cols_il(l, st12, arin12, 4 * l)
        artot12 = emit_allreduce(12, arin12, 8)

        z3 = big.tile([128, B_LOC, 2, 256], BF16, tag="tpB")
        st3 = small.tile([128, B_LOC, 2, 6], F32, tag="st3")
        for b in range(B_LOC):
            psz = ps.tile([128, 2, 256], F32, tag="ps", name=f"pz3{b}")
            for h in range(2):
                for dc in range(2):
                    nc.tensor.matmul(
                        out=psz[:, h, :],
                        lhsT=xT[:, dc, b * 256 + h * 128: b * 256 + (h + 1) * 128],
                        rhs=w123[:, dc, 512:768],
                        start=(dc == 0), stop=(dc == 1))
            nc.any.tensor_copy(out=z3[:, b, :, :], in_=psz[:])
            for h in range(2):
                nc.vector.bn_stats(out=st3[:, b, h, :], in_=z3[:, b, h, :])
        arin3 = small.tile([128, 4], F32, tag="arin3")
        stats_cols(2, st3, arin3, 0)
        artot3 = emit_allreduce(3, arin3, 4)

        s_l, bst_l = [None] * 3, [None] * 3
        s_l[0], bst_l[0] = bn_finalize(0, artot12[:, 0:4])
        s_l[1], bst_l[1] = bn_finalize(1, artot12[:, 4:8])
        s_l[2], bst_l[2] = bn_finalize(2, artot3)

        # ---- apply BN+ReLU; x1,x2 transposed (relu fused into psum copy), --
        # ---- x3 kept token-major with an all-ones column for softmax sums  --
        x1T = big.tile([128, 2, T], BF16, tag="tp1")
        x2T = big.tile([128, 2, T], BF16, tag="tpE")
        x3a = big.tile([128, B_LOC, 2, 260], BF16, tag="tpA")
        psa_cm = tc.tile_pool(name="psa", bufs=6, space="PSUM")
        psa = psa_cm.__enter__()
        # x1/x2: BN affine fused into the transpose matmuls:
        #   xT-block = z.T @ diag(s) + bst.T @ I   (relu rides the psum copy)
        dg = small.tile([128, 2, 2, 128], BF16, tag="dg")   # (layer, half, diag)
        for l in range(2):
            for h in range(2):
                nc.vector.tensor_scalar_mul(dg[:, l, h, :], idn[:],
                                            s_l[l][:, h:h + 1])
        for l, xiT in ((0, x1T), (1, x2T)):
            for b in range(B_LOC):
                pst = psa.tile([128, 2, 2, 128], F32, tag="psa",
                               name=f"pt{l}_{b}")
                for h in range(2):
                    for dc in range(2):
                        nc.tensor.matmul(
                            out=pst[:, dc, h, :],
                            lhsT=z12[:, b, h, 2 * dc * 128 + l: 2 * (dc + 1) * 128: 2],
                            rhs=dg[:, l, h, :],
                            start=True, stop=False)
                        nc.tensor.matmul(
                            out=pst[:, dc, h, :],
                            lhsT=bst_l[l][:, h, dc * 128:(dc + 1) * 128],
                            rhs=idn[:],
                            start=False, stop=True)
                nc.scalar.activation(
                    out=xiT[:, :, b * 256:(b + 1) * 256],
                    in_=pst[:].rearrange("p dc h t -> p dc (h t)"),
                    func=mybir.ActivationFunctionType.Relu)
        # x3: everything below runs strictly between AR3 and AR4 -> Pool is safe
        nc.vector.memset(x3a[:, :, :, 256:257], 1.0)
        for b in range(B_LOC):
            stg = stage.tile([128, 2, 256], BF16, tag="app2", name=f"ap2_{b}")
            for h in range(2):
                nc.vector.scalar_tensor_tensor(
                    out=stg[:, h, :], in0=z3[:, b, h, :],
                    scalar=s_l[2][:, h:h + 1], in1=bst_l[2][:, h, :],
                    op0=mybir.AluOpType.mult, op1=mybir.AluOpType.add)
            GPC.tensor_scalar_max(x3a[:, b, :, 0:256], stg[:], 0.0)

        # ---- attention + layer 4 -------------------------------------------
        z4 = big.tile([128, B_LOC, 2, 260], BF16, tag="tpB")  # 256 z | 257th wsum
        stats4 = small.tile([128, B_LOC, 2, 6], F32, tag="st4")
        for b in range(B_LOC):
            # S^T[m, n] per batch (exp via ACT; logits <= ~7, no max needed)
            pss = psa.tile([128, 2, 256], F32, tag="psa")      # (mchunk, n)
            for mc in range(2):
                for ec in range(2):
                    nc.tensor.matmul(
                        out=pss[:, mc, :],
                        lhsT=x2T[:, ec, b * 256 + mc * 128: b * 256 + (mc + 1) * 128],
                        rhs=x1T[:, ec, b * 256:(b + 1) * 256],
                        start=(ec == 0), stop=(ec == 1))
            pt = att.tile([128, 2, 256], BF16, tag="pt")   # exp(S^T/16)
            nc.scalar.activation(out=pt[:], in_=pss[:], scale=1.0 / 16.0,
                                 func=mybir.ActivationFunctionType.Exp)
            # AV with ones column -> per-token row sums in psum col 256;
            # normalize on the ACT copy (per-partition scale)
            rst = att.tile([128, 2, 256], BF16, tag="rst")  # (nchunk, d)
            for nc_ in range(2):
                psr = psa.tile([128, 260], F32, tag="psa", name=f"pr{b}_{nc_}")
                for mc in range(2):
                    nc.tensor.matmul(
                        out=psr[:, 0:257],
                        lhsT=pt[:, mc, nc_ * 128:(nc_ + 1) * 128],
                        rhs=x3a[:, b, mc, 0:257],
                        start=(mc == 0), stop=(mc == 1))
                invr = att.tile([128, 1], F32, tag="invr", name=f"iv{b}_{nc_}")
                nc.vector.reciprocal(out=invr[:], in_=psr[:, 256:257])
                nc.scalar.activation(out=rst[:, nc_, :], in_=psr[:, 0:256],
                                     scale=invr[:, 0:1],
                                     func=mybir.ActivationFunctionType.Copy)
            # transpose r -> [d, n]
            psrt = psa.tile([128, 2, 2, 128], BF16, tag="psa")  # (dc, nchunk, t)
            for nc_ in range(2):
                for dc in range(2):
                    nc.tensor.transpose(
                        out=psrt[:, dc, nc_, :],
                        in_=rst[:, nc_, dc * 128:(dc + 1) * 128],
                        identity=idn[:])
            rT = att.tile([128, 2, 256], BF16, tag="rT")
            nc.any.tensor_copy(out=rT[:],
                               in_=psrt[:].rearrange("p dc n t -> p dc (n t)"))
            # layer 4 with extra wb4 column (exact sync-BN E[y*b] term)
            for h in range(2):
                psy = psa.tile([128, 260], F32, tag="psa", name=f"py{b}_{h}")
                for dc in range(2):
                    nc.tensor.matmul(
                        out=psy[:, 0:257],
                        lhsT=rT[:, dc, h * 128:(h + 1) * 128],
                        rhs=w4[:, dc, 0:257],
                        start=(dc == 0), stop=(dc == 1))
                nc.any.tensor_copy(out=z4[:, b, h, 0:257], in_=psy[:, 0:257])
                nc.vector.bn_stats(out=stats4[:, b, h, :], in_=z4[:, b, h, 0:256])

        psa_cm.__exit__(None, None, None)
        # ---- final BN: exact stats all-reduce, apply, relu, store ----------
        arin4 = small.tile([128, 6], F32, tag="arin4")
        stats_cols(4, stats4, arin4, 0)
        for h in range(2):
            nc.vector.tensor_reduce(out=arin4[:, 4 + h:5 + h],
                                    in_=z4[:, :, h, 256:257],
                                    axis=mybir.AxisListType.XY,
                                    op=mybir.AluOpType.add)
        artot4 = emit_allreduce(4, arin4, 6)
        s4, bst4 = bn_finalize(3, artot4, wterm=artot4[:, 4:6])
        # post-AR4: Pool is collective-free again
        for b in range(B_LOC):
            ost = stage.tile([128, 2, 256], F32, tag="ost", name=f"os{b}")
            orl = stage.tile([128, 2, 256], F32, tag="orl", name=f"or{b}")
            for h in range(2):
                nc.vector.scalar_tensor_tensor(
                    out=ost[:, h, :], in0=z4[:, b, h, 0:256],
                    scalar=s4[:, h:h + 1], in1=bst4[:, h, :],
                    op0=mybir.AluOpType.mult, op1=mybir.AluOpType.add)
            GPC.tensor_scalar_max(orl[:], ost[:], 0.0)
            nc.sync.dma_start(out=out_r[:, b, :, :], in_=orl[:])

    _legalize_waits(nc)
    return nc


_CACHE = {}


def _prep_core_inputs(inputs):
    bf = ml_dtypes.bfloat16
    W = [inputs["W1"], inputs["W2"], inputs["W3"], inputs["W4"]]
    bs = [inputs["b1"], inputs["b2"], inputs["b3"], inputs["b4"]]
    gamma, beta = inputs["gamma"], inputs["beta"]

    w123 = np.zeros((128, 2, 768), dtype=bf)
    for c in range(2):
        w123[:, c, 0:512:2] = W[0][:, c * 128:(c + 1) * 128].T.astype(bf)
        w123[:, c, 1:512:2] = W[1][:, c * 128:(c + 1) * 128].T.astype(bf)
        w123[:, c, 512:768] = W[2][:, c * 128:(c + 1) * 128].T.astype(bf)
    w4 = np.zeros((128, 2, 260), dtype=bf)
    wb4 = (W[3].T.astype(np.float64) @ bs[3].astype(np.float64)).astype(np.float32)
    for c in range(2):
        w4[:, c, 0:256] = W[3][:, c * 128:(c + 1) * 128].T.astype(bf)
        w4[:, c, 256] = wb4[c * 128:(c + 1) * 128].astype(bf)
    bb = np.broadcast_to(np.stack(bs, 0)[None], (128, 4, 256)).astype(ml_dtypes.bfloat16)
    bb = np.ascontiguousarray(bb)
    gb = np.zeros((128, 2, 2), dtype=np.float32)
    for h in range(2):
        gb[:, h, 0] = gamma[h * 128:(h + 1) * 128]
        gb[:, h, 1] = beta[h * 128:(h + 1) * 128]
    hc = np.zeros((128, 8), dtype=np.float32)
    for l in range(4):
        hc[:, l] = bs[l].mean(dtype=np.float64)
    for l in range(3):
        hc[:, 4 + l] = (bs[l].astype(np.float64) ** 2).mean() - \
            bs[l].mean(dtype=np.float64) ** 2
    hc[:, 7] = (bs[3].astype(np.float64) ** 2).mean()
    return w123, w4, bb, gb, hc


def kernel(**inputs):
    _install_profile_shim()
    from concourse.bass_utils import run_bass_kernel_spmd

    if "nc" not in _CACHE:
        _CACHE["nc"] = build_program()
    nc = _CACHE["nc"]

    x = np.asarray(inputs["x"], dtype=np.float32)
    w123, w4, bb, gb, hc = _prep_core_inputs(
        {k: np.asarray(v) for k, v in inputs.items()})

    in_maps = []
    for i in range(NCORES):
        xs = np.ascontiguousarray(
            x[i * B_LOC:(i + 1) * B_LOC].reshape(T, 256))
        in_maps.append({"x": xs, "w123": w123, "w4": w4, "bb": bb,
                        "gb": gb, "hc": hc})

    trace = _CACHE.get("trace", False)
    res = run_bass_kernel_spmd(nc, in_maps, list(range(NCORES)), trace=trace)
    _CACHE["last_result"] = res

    out = np.empty((256, 256, 256), dtype=np.float32)
    for i in range(NCORES):
        out[i * B_LOC:(i + 1) * B_LOC] = res.results[i]["out"].reshape(
            B_LOC, 256, 256)
    return out



# revision 10
# speedup vs baseline: 1.5771x; 1.5771x over previous
"""Fused attention-block kernel for Trainium2, 8-core data-parallel over batch.

Computation (see harness reference): three BN+ReLU linear branches from the
same input, attention (QK^T/16 -> softmax -> AV), then a fourth BN+ReLU
linear.  BatchNorm1d is training-mode per-channel over (batch, feature) with
channel = sequence position, so batch-sharding needs a cross-core stats
all-reduce (sync-BN); weights are replicated.

Hardcoded: B=256, N=256, D=256, 8 cores -> 32 batches (8192 tokens) per core.

Structure (v2):
  - dummy AllReduce at t=0 absorbs the one-time collective-setup barrier
  - x cast+transpose pipelined in 8 chunks (separate DRAM tiles)
  - single merged stats AllReduce for layers 1/2/3
  - attention: softmax row-sums via pt.T @ ones matmuls (lands token-major),
    r^T computed directly as x3.T @ P^T (no PE transposes of r, no ACT
    normalize pass); 1/rowsum folded into the z4 psum evacuation
"""
import sys
import types

sys.path.insert(0, "/opt/trn_rl_repo")

import numpy as np
import ml_dtypes
from contextlib import ExitStack

import concourse.bass as bass
import concourse.mybir as mybir
import concourse.tile as tile
from concourse.masks import make_identity

BF16 = mybir.dt.bfloat16
F32 = mybir.dt.float32
NCORES = 8
B_LOC = 32          # batches per core
T = B_LOC * 256     # tokens per core
EPS = 1e-5


def _install_profile_shim():
    """run_bass_kernel_spmd(trace=True) under axon needs antenv.axon_hooks,
    which this image lacks; synthesize it (harmless if tracing unused)."""
    if "antenv.axon_hooks" in sys.modules:
        return
    try:
        import antenv
        mod = types.ModuleType("antenv.axon_hooks")
        mod._hook = None
        mod.set_axon_ntff_profile_hook = lambda h: setattr(mod, "_hook", h)
        mod.get_axon_ntff_profile_hook = lambda: mod._hook
        sys.modules["antenv.axon_hooks"] = mod
        antenv.axon_hooks = mod
        from trn_agent_boot.trn_boot import _ntff_profile_via_ctypes
        hook = _ntff_profile_via_ctypes("/opt/axon/libaxon_pjrt.so")
        if hook is not None:
            mod.set_axon_ntff_profile_hook(hook)
    except Exception:
        pass


def _legalize_waits(nc, max_waits=1):
    """HW instructions carry one sync-wait slot; walrus rejects instructions
    with too many waits.  Hoist extras onto engine-matched NoOps."""
    for f in nc.m.functions:
        for bb in f.blocks:
            insts = bb.instructions
            new_list = []
            for inst in insts:
                si = inst.sync_info
                if si is not None and len(si.on_wait) > max_waits:
                    waits = list(si.on_wait)
                    extra, keep = waits[:-max_waits], waits[-max_waits:]
                    for j, w in enumerate(extra):
                        nop = mybir.InstNoOp(
                            name=f"{inst.name}-waitnop{j}",
                            engine=inst.engine,
                            ins=[], outs=[],
                            sync_info=mybir.SyncInfo(on_wait=[w], on_update=[]),
                        )
                        nc.register_instruction(nop, overwrite=True)
                        new_list.append(nop)
                    inst.sync_info = mybir.SyncInfo(
                        on_wait=keep, on_update=list(si.on_update))
                new_list.append(inst)
            del insts[:]
            for x in new_list:
                insts.append(x)


def build_program(use_collectives=True):
    nc = bass.Bass("TRN2", target_bir_lowering=False, debug=False,
                   num_devices=NCORES)

    x_d = nc.dram_tensor("x", [T, 256], F32, kind="ExternalInput")
    w123_d = nc.dram_tensor("w123", [128, 2, 768], BF16, kind="ExternalInput")
    w4_d = nc.dram_tensor("w4", [128, 2, 260], BF16, kind="ExternalInput")
    bb_d = nc.dram_tensor("bb", [128, 4, 256], BF16, kind="ExternalInput")
    gb_d = nc.dram_tensor("gb", [128, 2, 2], F32, kind="ExternalInput")
    hc_d = nc.dram_tensor("hc", [128, 8], F32, kind="ExternalInput")
    out_d = nc.dram_tensor("out", [T, 256], F32, kind="ExternalOutput")

    groups = [list(range(NCORES))]
    out_r = out_d.ap().rearrange("(b h p) e -> p b h e", b=B_LOC, h=2, p=128)

    with ExitStack() as ctx:
        tc = ctx.enter_context(tile.TileContext(nc))
        big = ctx.enter_context(tc.tile_pool(name="big", bufs=1))
        small = ctx.enter_context(tc.tile_pool(name="small", bufs=1))
        stage = ctx.enter_context(tc.tile_pool(name="stage", bufs=3))
        att = ctx.enter_context(tc.tile_pool(name="att", bufs=4))
        dram = ctx.enter_context(tc.tile_pool(name="dram", bufs=1, space="DRAM"))

        # ---- t=0: dummy collective to absorb one-time setup/barrier --------
        dmy = small.tile([128, 1], F32, tag="dmy")
        nc.vector.memset(dmy[:], 0.0)
        ar0_i = dram.tile([128, 1], F32, tag="ar0i")
        ar0_o = dram.tile([128, 1], F32, tag="ar0o")
        nc.sync.dma_start(out=ar0_i[:], in_=dmy[:])
        if use_collectives:
            nc.gpsimd.collective_compute(
                "AllReduce", mybir.AluOpType.add, replica_groups=groups,
                ins=[ar0_i[:].opt()], outs=[ar0_o[:].opt()])
        else:
            nc.gpsimd.dma_start(out=ar0_o[:], in_=ar0_i[:])

        # preload ACT tables off the critical path: exp set, then sqrt set
        # (leaves sqrt set resident for the post-AR finalize; relu/copy are
        # in every set)
        dmy2 = small.tile([128, 1], F32, tag="dmy2")
        nc.scalar.activation(out=dmy2[:], in_=dmy[:],
                             func=mybir.ActivationFunctionType.Exp)
        nc.scalar.sqrt(out=dmy2[:], in_=dmy[:])

        # ---- constant loads -------------------------------------------------
        w123 = small.tile([128, 2, 768], BF16, tag="w123")
        w4 = small.tile([128, 2, 260], BF16, tag="w4")
        bbt = small.tile([128, 4, 256], BF16, tag="bbt")
        gbt = small.tile([128, 2, 2], F32, tag="gbt")
        hct = small.tile([128, 8], F32, tag="hct")
        idn = small.tile([128, 128], BF16, tag="idn")
        ones1 = small.tile([128, 1], BF16, tag="ones1")
        nc.sync.dma_start(out=w123[:], in_=w123_d.ap())
        nc.sync.dma_start(out=w4[:], in_=w4_d.ap())
        nc.sync.dma_start(out=bbt[:], in_=bb_d.ap())
        nc.sync.dma_start(out=gbt[:], in_=gb_d.ap())
        nc.sync.dma_start(out=hct[:], in_=hc_d.ap())
        make_identity(nc, idn[:])
        nc.vector.memset(ones1[:], 1.0)

        # ---- x -> bf16 (SWDGE cast) chunked, pipelined with xbar transposes
        NXC = 8
        CH = T // NXC                                    # 1024 tokens/chunk
        xT = big.tile([128, 2, T], BF16, tag="xT")       # (dchunk, token)
        xbf = []
        for c in range(NXC):
            xb = dram.tile([CH, 256], BF16, tag=f"xbf{c}", name=f"xb{c}")
            nc.gpsimd.dma_start(out=xb[:],
                                in_=x_d.ap()[c * CH:(c + 1) * CH, :])
            xbf.append(xb)
        for c in range(NXC):
            for dc in range(2):
                nc.sync.dma_start_transpose(
                    out=xT[:, dc, c * CH:(c + 1) * CH],
                    in_=xbf[c][:, dc * 128:(dc + 1) * 128])

        # ---- helper: per-layer BN scale/shift from all-reduced stats --------
        def bn_finalize(lidx, artot, wterm=None):
            """artot [128,4] per half h: cols 2h = global mean, 2h+1 = global
            E[y^2] (already divided by NCORES).  Returns (s f32, bst bf16)."""
            meany = small.tile([128, 2], F32, tag=f"meany{lidx}", name=f"my{lidx}")
            ey2 = small.tile([128, 2], F32, tag=f"ey2{lidx}", name=f"ey{lidx}")
            nc.vector.tensor_scalar_mul(meany[:], artot[:, 0:4:2], 1.0 / NCORES)
            nc.vector.tensor_scalar_mul(ey2[:], artot[:, 1:4:2], 1.0 / NCORES)
            meanz = small.tile([128, 2], F32, tag=f"meanz{lidx}", name=f"mz{lidx}")
            nc.vector.tensor_scalar_add(meanz[:], meany[:], hct[:, lidx:lidx + 1])
            varz = small.tile([128, 2], F32, tag=f"varz{lidx}", name=f"vz{lidx}")
            m2 = small.tile([128, 2], F32, tag=f"m2_{lidx}", name=f"m2{lidx}")
            if wterm is not None:
                # exact: E[z^2] = E[y^2] + 2 E[y b] + mean(b^2)
                eyb = small.tile([128, 2], F32, tag=f"eyb{lidx}", name=f"eb{lidx}")
                nc.vector.tensor_scalar_mul(eyb[:], wterm[:], 2.0 / 65536.0)
                nc.vector.tensor_tensor(out=ey2[:], in0=ey2[:], in1=eyb[:],
                                        op=mybir.AluOpType.add)
                nc.vector.tensor_scalar_add(ey2[:], ey2[:], hct[:, 7:8])
                nc.vector.tensor_tensor(out=m2[:], in0=meanz[:], in1=meanz[:],
                                        op=mybir.AluOpType.mult)
                nc.vector.tensor_tensor(out=varz[:], in0=ey2[:], in1=m2[:],
                                        op=mybir.AluOpType.subtract)
                nc.vector.tensor_scalar_add(varz[:], varz[:], EPS)
            else:
                # var_z ~= var_y + var(b) (bias covariance negligible here)
                nc.vector.tensor_tensor(out=m2[:], in0=meany[:], in1=meany[:],
                                        op=mybir.AluOpType.mult)
                nc.vector.tensor_tensor(out=varz[:], in0=ey2[:], in1=m2[:],
                                        op=mybir.AluOpType.subtract)
                nc.vector.tensor_scalar(varz[:], varz[:],
                                        hct[:, 4 + lidx:5 + lidx], EPS,
                                        mybir.AluOpType.add,
                                        mybir.AluOpType.add)
            sd = small.tile([128, 2], F32, tag=f"sd{lidx}", name=f"sd{lidx}")
            nc.scalar.sqrt(out=sd[:], in_=varz[:])
            rstd = small.tile([128, 2], F32, tag=f"rstd{lidx}", name=f"rs{lidx}")
            nc.vector.reciprocal(out=rstd[:], in_=sd[:])
            s = small.tile([128, 2], F32, tag=f"s{lidx}", name=f"s{lidx}")
            nc.vector.tensor_tensor(out=s[:], in0=rstd[:], in1=gbt[:, :, 0],
                                    op=mybir.AluOpType.mult)
            tsh = small.tile([128, 2], F32, tag=f"tsh{lidx}", name=f"t{lidx}")
            nc.vector.tensor_tensor(out=tsh[:], in0=meanz[:], in1=s[:],
                                    op=mybir.AluOpType.mult)
            nc.vector.tensor_tensor(out=tsh[:], in0=gbt[:, :, 1], in1=tsh[:],
                                    op=mybir.AluOpType.subtract)
            bst = small.tile([128, 2, 256], BF16, tag=f"bst{lidx}", name=f"b{lidx}")
            for h in range(2):
                nc.vector.tensor_scalar(bst[:, h, :], bbt[:, lidx, :],
                                        s[:, h:h + 1], tsh[:, h:h + 1],
                                        mybir.AluOpType.mult,
                                        mybir.AluOpType.add)
            return s, bst

        def emit_allreduce(lidx, arin, width):
            ar_i = dram.tile([128, width], F32, tag=f"ari{lidx}", name=f"ai{lidx}")
            ar_o = dram.tile([128, width], F32, tag=f"aro{lidx}", name=f"ao{lidx}")
            nc.sync.dma_start(out=ar_i[:], in_=arin[:])
            if use_collectives:
                nc.gpsimd.collective_compute(
                    "AllReduce", mybir.AluOpType.add, replica_groups=groups,
                    ins=[ar_i[:].opt()], outs=[ar_o[:].opt()])
            else:
                nc.gpsimd.dma_start(out=ar_o[:], in_=ar_i[:])
            artot = small.tile([128, width], F32, tag=f"artot{lidx}",
                               name=f"at{lidx}")
            nc.sync.dma_start(out=artot[:], in_=ar_o[:])
            return artot

        def stats_cols(lidx, stats, arin, col0):
            """stats: [128, B_LOC, 2, 6] per-batch bn_stats rows (each row =
            even/odd 3-tuples (count, mean, count*var) x2, both halves are the
            same layer).  arin[:, col0+2h] = core mean, +1 = core E[y^2]."""
            for h in range(2):
                msum = small.tile([128, 1], F32, tag=f"ms{lidx}_{h}",
                                  name=f"ms{lidx}{h}")
                nc.vector.tensor_reduce(out=msum[:], in_=stats[:, :, h, 1:5:3],
                                        axis=mybir.AxisListType.XY,
                                        op=mybir.AluOpType.add)
                sq = small.tile([128, B_LOC, 2], F32, tag=f"sq{lidx}_{h}",
                                name=f"sq{lidx}{h}")
                nc.vector.tensor_tensor(out=sq[:], in0=stats[:, :, h, 1:5:3],
                                        in1=stats[:, :, h, 1:5:3],
                                        op=mybir.AluOpType.mult)
                sqsum = small.tile([128, 1], F32, tag=f"qs{lidx}_{h}",
                                   name=f"qs{lidx}{h}")
                nc.vector.tensor_reduce(out=sqsum[:], in_=sq[:],
                                        axis=mybir.AxisListType.XY,
                                        op=mybir.AluOpType.add)
                cvsum = small.tile([128, 1], F32, tag=f"cv{lidx}_{h}",
                                   name=f"cv{lidx}{h}")
                nc.vector.tensor_reduce(out=cvsum[:], in_=stats[:, :, h, 2:6:3],
                                        axis=mybir.AxisListType.XY,
                                        op=mybir.AluOpType.add)
                nc.vector.tensor_scalar_mul(
                    arin[:, col0 + 2 * h:col0 + 2 * h + 1], msum[:],
                    1.0 / (2 * B_LOC))
                nc.vector.tensor_scalar_mul(cvsum[:], cvsum[:],
                                            1.0 / (128 * 2 * B_LOC))
                nc.vector.tensor_scalar_mul(
                    arin[:, col0 + 2 * h + 1:col0 + 2 * h + 2], sqsum[:],
                    1.0 / (2 * B_LOC))
                nc.vector.tensor_tensor(
                    out=arin[:, col0 + 2 * h + 1:col0 + 2 * h + 2],
                    in0=arin[:, col0 + 2 * h + 1:col0 + 2 * h + 2],
                    in1=cvsum[:], op=mybir.AluOpType.add)

        def stats_cols_il(lidx, st, arin, col0):
            """st: [128, B_LOC, 2, 6]; z1 occupies even elements (cols 0:3),
            z2 odd (cols 3:6) of each interleaved 512-row.  Per half h:
            arin[:, col0+2h] = core mean, +1 = core E[y^2]."""
            for h in range(2):
                msum = small.tile([128, 1], F32, tag=f"ms{lidx}_{h}",
                                  name=f"ms{lidx}{h}")
                nc.vector.tensor_reduce(
                    out=msum[:], in_=st[:, :, h, 3 * lidx + 1:3 * lidx + 2],
                    axis=mybir.AxisListType.XY, op=mybir.AluOpType.add)
                sq = small.tile([128, B_LOC, 1], F32, tag=f"sq{lidx}_{h}",
                                name=f"sq{lidx}{h}")
                nc.vector.tensor_tensor(
                    out=sq[:], in0=st[:, :, h, 3 * lidx + 1:3 * lidx + 2],
                    in1=st[:, :, h, 3 * lidx + 1:3 * lidx + 2],
                    op=mybir.AluOpType.mult)
                sqsum = small.tile([128, 1], F32, tag=f"qs{lidx}_{h}",
                                   name=f"qs{lidx}{h}")
                nc.vector.tensor_reduce(out=sqsum[:], in_=sq[:],
                                        axis=mybir.AxisListType.XY,
                                        op=mybir.AluOpType.add)
                cvsum = small.tile([128, 1], F32, tag=f"cv{lidx}_{h}",
                                   name=f"cv{lidx}{h}")
                nc.vector.tensor_reduce(
                    out=cvsum[:], in_=st[:, :, h, 3 * lidx + 2:3 * lidx + 3],
                    axis=mybir.AxisListType.XY, op=mybir.AluOpType.add)
                nc.vector.tensor_scalar_mul(
                    arin[:, col0 + 2 * h:col0 + 2 * h + 1], msum[:], 1.0 / B_LOC)
                nc.vector.tensor_scalar_mul(cvsum[:], cvsum[:],
                                            1.0 / (256 * B_LOC))
                nc.vector.tensor_scalar_mul(
                    arin[:, col0 + 2 * h + 1:col0 + 2 * h + 2], sqsum[:],
                    1.0 / B_LOC)
                nc.vector.tensor_tensor(
                    out=arin[:, col0 + 2 * h + 1:col0 + 2 * h + 2],
                    in0=arin[:, col0 + 2 * h + 1:col0 + 2 * h + 2],
                    in1=cvsum[:], op=mybir.AluOpType.add)

        # ---- layers 1+2 (one 512-wide interleaved matmul) + layer 3 ---------
        # per batch: one merged loop so lhsT loads are shared and PE stays hot
        z12 = big.tile([128, B_LOC, 2, 512], BF16, tag="z12")   # (b, h, z1|z2)
        # allocated one col wider than needed so tag "z3" can host z4 later
        z3 = big.tile([128, B_LOC, 2, 257], BF16, tag="z3")
        st12 = small.tile([128, B_LOC, 2, 6], F32, tag="st12")
        st3 = small.tile([128, B_LOC, 2, 6], F32, tag="st3")
        ps12_cm = tc.tile_pool(name="ps12", bufs=2, space="PSUM")
        ps12 = ps12_cm.__enter__()
        ps3_cm = tc.tile_pool(name="ps3", bufs=2, space="PSUM")
        ps3 = ps3_cm.__enter__()
        for b in range(B_LOC):
            p12 = ps12.tile([128, 2, 512], F32, tag="p12", name=f"pz{b}")
            p3 = ps3.tile([128, 2, 256], F32, tag="p3", name=f"pz3{b}")
            for h in range(2):
                tok = slice(b * 256 + h * 128, b * 256 + (h + 1) * 128)
                for dc in range(2):
                    nc.tensor.matmul(
                        out=p12[:, h, :], lhsT=xT[:, dc, tok],
                        rhs=w123[:, dc, 0:512],
                        start=(dc == 0), stop=(dc == 1))
                for dc in range(2):
                    nc.tensor.matmul(
                        out=p3[:, h, :], lhsT=xT[:, dc, tok],
                        rhs=w123[:, dc, 512:768],
                        start=(dc == 0), stop=(dc == 1))
            # evac on ACT (copy is in every table set), stats on DVE from PSUM
            nc.scalar.copy(out=z12[:, b, :, :], in_=p12[:])
            nc.vector.tensor_copy(out=z3[:, b, :, 0:256], in_=p3[:])
            for h in range(2):
                nc.vector.bn_stats(out=st12[:, b, h, :], in_=p12[:, h, :])
                nc.vector.bn_stats(out=st3[:, b, h, :], in_=p3[:, h, :])
        ps3_cm.__exit__(None, None, None)
        ps12_cm.__exit__(None, None, None)

        # ---- one merged AllReduce for L1/L2/L3 stats ------------------------
        arin = small.tile([128, 12], F32, tag="arin")
        for l in range(2):
            stats_cols_il(l, st12, arin, 4 * l)
        stats_cols(2, st3, arin, 8)
        artot = emit_allreduce(123, arin, 12)

        s_l, bst_l = [None] * 3, [None] * 3
        s_l[0], bst_l[0] = bn_finalize(0, artot[:, 0:4])
        s_l[1], bst_l[1] = bn_finalize(1, artot[:, 4:8])
        s_l[2], bst_l[2] = bn_finalize(2, artot[:, 8:12])

        # ---- apply BN+ReLU; x1,x2 via fused BN+transpose matmuls ------------
        x1T = big.tile([128, 2, T], BF16, tag="xT")      # reuse xT space
        x2T = big.tile([128, 2, T], BF16, tag="x2T")
        x3a = big.tile([128, B_LOC, 2, 256], BF16, tag="z12")  # reuse z12 space
        psa_cm = tc.tile_pool(name="psa", bufs=6, space="PSUM")
        psa = psa_cm.__enter__()
        # x1/x2: xT-block = z.T @ diag(s) + bst.T @ I (relu rides psum copy)
        dg = small.tile([128, 2, 2, 128], BF16, tag="dg")   # (layer, half, diag)
        for l in range(2):
            for h in range(2):
                nc.vector.tensor_scalar_mul(dg[:, l, h, :], idn[:],
                                            s_l[l][:, h:h + 1])
        for l, xiT in ((0, x1T), (1, x2T)):
            for b in range(B_LOC):
                pst = psa.tile([128, 2, 2, 128], F32, tag="psa",
                               name=f"pt{l}_{b}")
                for h in range(2):
                    for dc in range(2):
                        nc.tensor.matmul(
                            out=pst[:, dc, h, :],
                            lhsT=z12[:, b, h, 2 * dc * 128 + l: 2 * (dc + 1) * 128: 2],
                            rhs=dg[:, l, h, :],
                            start=True, stop=False)
                        nc.tensor.matmul(
                            out=pst[:, dc, h, :],
                            lhsT=bst_l[l][:, h, dc * 128:(dc + 1) * 128],
                            rhs=idn[:],
                            start=False, stop=True)
                # alternate evacuation engine to balance ACT/DVE load
                if b % 2 == 0:
                    nc.scalar.activation(
                        out=xiT[:, :, b * 256:(b + 1) * 256],
                        in_=pst[:].rearrange("p dc h t -> p dc (h t)"),
                        func=mybir.ActivationFunctionType.Relu)
                else:
                    nc.vector.tensor_scalar_max(
                        xiT[:, :, b * 256:(b + 1) * 256].rearrange(
                            "p dc (h t) -> p dc h t", h=2),
                        pst[:], 0.0)
        # x3: token-major BN apply (scale/shift per position) + relu
        for b in range(B_LOC):
            stg = stage.tile([128, 2, 256], BF16, tag="app2", name=f"ap2_{b}")
            for h in range(2):
                nc.vector.scalar_tensor_tensor(
                    out=stg[:, h, :], in0=z3[:, b, h, 0:256],
                    scalar=s_l[2][:, h:h + 1], in1=bst_l[2][:, h, :],
                    op0=mybir.AluOpType.mult, op1=mybir.AluOpType.add)
            nc.vector.tensor_scalar_max(x3a[:, b, :, :], stg[:], 0.0)
        psa_cm.__exit__(None, None, None)

        # ---- attention + layer 4 -------------------------------------------
        # per batch: S^T -> exp -> (row sums via pt.T @ ones; r^T directly via
        # x3.T @ P^T) -> L4 with 1/rowsum folded into the psum evacuation
        z4 = big.tile([128, B_LOC, 2, 257], BF16, tag="z3")   # reuse z3 space
        stats4 = small.tile([128, B_LOC, 2, 6], F32, tag="st4")
        pss_cm = tc.tile_pool(name="pss", bufs=2, space="PSUM")
        pss_p = pss_cm.__enter__()
        prt_cm = tc.tile_pool(name="prt", bufs=2, space="PSUM")
        prt_p = prt_cm.__enter__()
        pz4_cm = tc.tile_pool(name="pz4", bufs=2, space="PSUM")
        pz4_p = pz4_cm.__enter__()
        for b in range(B_LOC):
            # S^T[m, n] per batch (exp via ACT; logits <= ~7, no max needed)
            pss = pss_p.tile([128, 2, 256], F32, tag="pss", name=f"ps{b}")
            for mc in range(2):
                for ec in range(2):
                    nc.tensor.matmul(
                        out=pss[:, mc, :],
                        lhsT=x2T[:, ec, b * 256 + mc * 128: b * 256 + (mc + 1) * 128],
                        rhs=x1T[:, ec, b * 256:(b + 1) * 256],
                        start=(ec == 0), stop=(ec == 1))
            pt = att.tile([128, 2, 256], BF16, tag="pt")   # exp(S^T/16)
            nc.scalar.activation(out=pt[:], in_=pss[:], scale=1.0 / 16.0,
                                 func=mybir.ActivationFunctionType.Exp)
            # z4 psum also holds the softmax row sums in cols 258:259
            pz4 = pz4_p.tile([128, 2, 260], F32, tag="pz4", name=f"pz4_{b}")
            for nc_ in range(2):
                for mc in range(2):
                    nc.tensor.matmul(
                        out=pz4[:, nc_, 258:259],
                        lhsT=pt[:, mc, nc_ * 128:(nc_ + 1) * 128],
                        rhs=ones1[:],
                        start=(mc == 0), stop=(mc == 1))
            invr = att.tile([128, 2], F32, tag="invr", name=f"iv{b}")
            nc.vector.reciprocal(out=invr[:], in_=pz4[:, :, 258])
            # r^T = x3.T @ P^T  (unnormalized), accumulated over m-chunks
            prt = prt_p.tile([128, 2, 256], F32, tag="prt", name=f"pr{b}")
            for dc in range(2):
                for mc in range(2):
                    nc.tensor.matmul(
                        out=prt[:, dc, :],
                        lhsT=x3a[:, b, mc, dc * 128:(dc + 1) * 128],
                        rhs=pt[:, mc, :],
                        start=(mc == 0), stop=(mc == 1))
            rT = att.tile([128, 2, 256], BF16, tag="rT", name=f"rt{b}")
            nc.scalar.copy(out=rT[:], in_=prt[:])
            # layer 4 with extra wb4 column (exact sync-BN E[y*b] term)
            for hn in range(2):
                for dc in range(2):
                    nc.tensor.matmul(
                        out=pz4[:, hn, 0:257],
                        lhsT=rT[:, dc, hn * 128:(hn + 1) * 128],
                        rhs=w4[:, dc, 0:257],
                        start=(dc == 0), stop=(dc == 1))
            for hn in range(2):
                nc.vector.tensor_scalar_mul(z4[:, b, hn, :],
                                            pz4[:, hn, 0:257],
                                            invr[:, hn:hn + 1])
                nc.vector.bn_stats(out=stats4[:, b, hn, :],
                                   in_=z4[:, b, hn, 0:256])
        pz4_cm.__exit__(None, None, None)
        prt_cm.__exit__(None, None, None)
        pss_cm.__exit__(None, None, None)

        # ---- final BN: exact stats all-reduce, apply, relu, store ----------
        arin4 = small.tile([128, 6], F32, tag="arin4")
        stats_cols(4, stats4, arin4, 0)
        for h in range(2):
            nc.vector.tensor_reduce(out=arin4[:, 4 + h:5 + h],
                                    in_=z4[:, :, h, 256:257],
                                    axis=mybir.AxisListType.XY,
                                    op=mybir.AluOpType.add)
        artot4 = emit_allreduce(4, arin4, 6)
        s4, bst4 = bn_finalize(3, artot4, wterm=artot4[:, 4:6])
        for b in range(B_LOC):
            ost = stage.tile([128, 2, 256], F32, tag="ost", name=f"os{b}")
            orl = stage.tile([128, 2, 256], F32, tag="orl", name=f"or{b}")
            for h in range(2):
                nc.vector.scalar_tensor_tensor(
                    out=ost[:, h, :], in0=z4[:, b, h, 0:256],
                    scalar=s4[:, h:h + 1], in1=bst4[:, h, :],
                    op0=mybir.AluOpType.mult, op1=mybir.AluOpType.add)
            nc.vector.tensor_scalar_max(orl[:], ost[:], 0.0)
            nc.sync.dma_start(out=out_r[:, b, :, :], in_=orl[:])

    _legalize_waits(nc)
    return nc


_CACHE = {}


def _prep_core_inputs(inputs):
    bf = ml_dtypes.bfloat16
    W = [inputs["W1"], inputs["W2"], inputs["W3"], inputs["W4"]]
    bs = [inputs["b1"], inputs["b2"], inputs["b3"], inputs["b4"]]
    gamma, beta = inputs["gamma"], inputs["beta"]

    w123 = np.zeros((128, 2, 768), dtype=bf)
    for c in range(2):
        w123[:, c, 0:512:2] = W[0][:, c * 128:(c + 1) * 128].T.astype(bf)
        w123[:, c, 1:512:2] = W[1][:, c * 128:(c + 1) * 128].T.astype(bf)
        w123[:, c, 512:768] = W[2][:, c * 128:(c + 1) * 128].T.astype(bf)
    w4 = np.zeros((128, 2, 260), dtype=bf)
    wb4 = (W[3].T.astype(np.float64) @ bs[3].astype(np.float64)).astype(np.float32)
    for c in range(2):
        w4[:, c, 0:256] = W[3][:, c * 128:(c + 1) * 128].T.astype(bf)
        w4[:, c, 256] = wb4[c * 128:(c + 1) * 128].astype(bf)
    bb = np.broadcast_to(np.stack(bs, 0)[None], (128, 4, 256)).astype(ml_dtypes.bfloat16)
    bb = np.ascontiguousarray(bb)
    gb = np.zeros((128, 2, 2), dtype=np.float32)
    for h in range(2):
        gb[:, h, 0] = gamma[h * 128:(h + 1) * 128]
        gb[:, h, 1] = beta[h * 128:(h + 1) * 128]
    hc = np.zeros((128, 8), dtype=np.float32)
    for l in range(4):
        hc[:, l] = bs[l].mean(dtype=np.float64)
    for l in range(3):
        hc[:, 4 + l] = (bs[l].astype(np.float64) ** 2).mean() - \
            bs[l].mean(dtype=np.float64) ** 2
    hc[:, 7] = (bs[3].astype(np.float64) ** 2).mean()
    return w123, w4, bb, gb, hc


def kernel(**inputs):
    _install_profile_shim()
    from concourse.bass_utils import run_bass_kernel_spmd

    if "nc" not in _CACHE:
        _CACHE["nc"] = build_program()
    nc = _CACHE["nc"]

    x = np.asarray(inputs["x"], dtype=np.float32)
    w123, w4, bb, gb, hc = _prep_core_inputs(
        {k: np.asarray(v) for k, v in inputs.items()})

    in_maps = []
    for i in range(NCORES):
        xs = np.ascontiguousarray(
            x[i * B_LOC:(i + 1) * B_LOC].reshape(T, 256))
        in_maps.append({"x": xs, "w123": w123, "w4": w4, "bb": bb,
                        "gb": gb, "hc": hc})

    trace = _CACHE.get("trace", False)
    res = run_bass_kernel_spmd(nc, in_maps, list(range(NCORES)), trace=trace)
    _CACHE["last_result"] = res

    out = np.empty((256, 256, 256), dtype=np.float32)
    for i in range(NCORES):
        out[i * B_LOC:(i + 1) * B_LOC] = res.results[i]["out"].reshape(
            B_LOC, 256, 256)
    return out
